# revision 14
# baseline (speedup 1.0000x reference)
"""GRU decoder Bass kernel for Trainium2, data-parallel over batch on 8 cores.

Math refactoring (exactly equivalent to the reference up to fp assoc.):
  context = hidden[0] is constant across steps, and x_{t} = fc_out_{t-1} is
  linear in [h_t, context].  Folding fc into the input projection:
    gi_t = h_t @ M1.T + CONST          (M1 = W_ih @ fc_W[:, :H], t >= 1)
    gh_t = h_t @ W_hh.T + b_hh
  r/z gates add gi+gh, so P_r = M1_r + W_hh_r, P_z = M1_z + W_hh_z fuse into
  one [4096, 1024] weight:  G_t = h_t @ [P_r | P_z | M1_n | W_hh_n].T + C
  fc_out_t = h_{t+1} @ F1.T + CF with F1 = fc_W[:, :H].
  GRU update in "w-form": w = sigmoid(-pre_z) = 1-z, h' = h + w*(n - h).

Performance structure (v3):
  - per core B=64 rows, "split layout": [128 parts = 2 hidden-halves x 64
    batch, 512 free].  M=64 matmul pairs are col-tiled (auto tile_position
    (0,0)/(0,64)) and stream concurrently -> G = 16384 effective columns =
    6.8us/step, the fp16 1 col/cycle PE floor.
  - G banks ordered pre_r, h_n, i_n (2 N=256 halves), pre_z (2 halves), so
    the gate chain overlaps the stream; tail after the last matmul is just
    sigmoid(-pre_z) -> w*d -> h+., with d = n-h precomputed.
  - gate-bank constants are written into PSUM by the idle ACT/DVE engines
    (has_written bits remain set from the previous step's accumulation, so
    start=False matmuls accumulate on top); no PE init matmuls per step.
  - fp16 state + intermediates; fc deferred one step; 4x 128x128 fp16 PE
    transposes/step.  PE never idles -> HAM stays at 2.4 GHz.
"""
import os
import numpy as np

H = 1024
OUT = 768
BATCH = 512
NCORES = 8
B = BATCH // NCORES  # 64

_BUILD_CACHE = {}

# K-chunk m covers contraction dims offs(m) .. offs(m)+127
def _offs(m):
    return 128 * (m // 2) + 512 * (m % 2)


def _build(T: int):
    from contextlib import ExitStack
    from concourse import tile, mybir, bacc

    F16 = mybir.dt.float16
    F32 = mybir.dt.float32
    Sig = mybir.ActivationFunctionType.Sigmoid
    Tanh = mybir.ActivationFunctionType.Tanh
    Copy = mybir.ActivationFunctionType.Copy

    nc = bacc.Bacc("TRN2", target_bir_lowering=False, debug=False,
                   num_devices=NCORES)

    dram = {}
    def din(name, shape, dt):
        dram[name] = nc.dram_tensor(name, list(shape), dt, kind="ExternalInput").ap()
        return dram[name]

    w4_d = din("W4", [128, 8 * 4096], F16)
    f1_d = din("F1", [128, 8 * 768], F16)
    ci_d = din("CINIT", [128, 8 * 512], F16)
    cst_d = din("CST", [128, 4 * 512], F16)
    id2_d = din("IDENT2", [128, 64], F16)
    idt_d = din("IDENTT", [128, 128], F16)
    h0s_d = din("H0S", [128, 512], F16)
    h0t_d = din("H0T", [128, 8 * 64], F16)
    g0_d = din("G0", [128, 4 * 512], F32)
    cf_d = din("CF", [128, 384], F32)
    out_d = nc.dram_tensor("OUT", [T * 128, 384], F32, kind="ExternalOutput").ap()

    with tile.TileContext(nc) as tc:
        with ExitStack() as ctx:
            wpool = ctx.enter_context(tc.tile_pool(name="weights", bufs=1))
            state = ctx.enter_context(tc.tile_pool(name="state", bufs=2))
            tmp = ctx.enter_context(tc.tile_pool(name="tmp", bufs=2))
            gps = ctx.enter_context(tc.tile_pool(name="gpsum", bufs=1, space="PSUM"))
            fps = ctx.enter_context(tc.tile_pool(name="fpsum", bufs=2, space="PSUM"))
            tps = ctx.enter_context(tc.tile_pool(name="tpsum", bufs=2, space="PSUM"))

            w4 = wpool.tile([128, 8 * 4096], F16, name="w4")
            f1 = wpool.tile([128, 8 * 768], F16, name="f1")
            ci = wpool.tile([128, 8 * 512], F16, name="ci")
            cst = wpool.tile([128, 4 * 512], F16, name="cst")
            id2 = wpool.tile([128, 64], F16, name="id2")
            idt = wpool.tile([128, 128], F16, name="idt")
            g0 = wpool.tile([128, 4 * 512], F32, name="g0")
            cf = wpool.tile([128, 384], F32, name="cf")

            h0 = state.tile([128, 512], F16, name="h0", tag="h")
            hT0 = state.tile([128, 8 * 64], F16, name="hT0", tag="hT")

            for t_sb, t_d in ((w4, w4_d), (f1, f1_d), (ci, ci_d), (cst, cst_d),
                              (id2, id2_d), (idt, idt_d), (h0, h0s_d),
                              (hT0, h0t_d), (g0, g0_d), (cf, cf_d)):
                nc.sync.dma_start(t_sb[:], t_d[:])

            # 4 gate PSUM banks, jj: 0=pre_r, 1=pre_z, 2=i_n, 3=h_n
            gb = [gps.tile([128, 512], F32, name=f"gb{j}", tag=f"gb{j}")
                  for j in range(4)]

            def emit_init_mm(jjs):
                # PE-matmul constant init (prologue only): hi+lo fp16 rows
                for jj in jjs:
                    for g in range(2):
                        cc = jj * 2 + g
                        nc.tensor.matmul(
                            gb[jj][64 * g:64 * (g + 1), :], id2[:, :],
                            ci[:, cc * 512:(cc + 1) * 512],
                            start=True, stop=False)

            def emit_init_eng(jj):
                # ACT-written constants: has_written bits stay set from
                # the previous step's matmuls, so start=False MMs accumulate
                s = cst[:, jj * 512:(jj + 1) * 512]
                nc.scalar.activation(gb[jj][:, :], s, Copy)

            def emit_G_bank(hT, jj, c0, c1, skip_gc):
                for m in range(8):
                    lhsT = hT[:, m * 64:(m + 1) * 64]
                    for g in range(2):
                        cc = jj * 2 + g
                        nc.tensor.matmul(
                            gb[jj][64 * g:64 * (g + 1), c0:c1], lhsT,
                            w4[:, m * 4096 + cc * 512 + c0: m * 4096 + cc * 512 + c1],
                            start=False, stop=(m == 7),
                            skip_group_check=skip_gc)

            def emit_gates(t, h_prev, pr, pz_h, pin_h, phn, init_after_r=None):
                r = tmp.tile([128, 512], F16, name=f"r{t}", tag="r")
                t1 = tmp.tile([128, 512], F16, name=f"t1{t}", tag="t1")
                t2 = tmp.tile([128, 512], F16, name=f"t2{t}", tag="t2")
                n = tmp.tile([128, 512], F16, name=f"n{t}", tag="n")
                dd = tmp.tile([128, 512], F16, name=f"d{t}", tag="d")
                w = tmp.tile([128, 512], F16, name=f"w{t}", tag="w")
                v = tmp.tile([128, 512], F16, name=f"v{t}", tag="v")
                h_new = state.tile([128, 512], F16, name=f"h{t}", tag="h")

                nc.scalar.activation(r[:], pr, Sig)
                if init_after_r is not None:
                    emit_init_eng(init_after_r)
                nc.vector.tensor_mul(t1[:], r[:], phn)
                for hf in range(2):
                    s = slice(256 * hf, 256 * (hf + 1))
                    nc.vector.tensor_add(t2[:, s], t1[:, s], pin_h(hf))
                    nc.scalar.activation(n[:, s], t2[:, s], Tanh)
                    nc.vector.tensor_sub(dd[:, s], n[:, s], h_prev[:, s])
                    nc.scalar.activation(w[:, s], pz_h(hf), Sig, scale=-1.0)
                    nc.vector.tensor_mul(v[:, s], w[:, s], dd[:, s])
                    nc.vector.tensor_add(h_new[:, s], h_prev[:, s], v[:, s])
                return h_new

            def emit_transpose(t, h_new, hT_new, i, eng="dve"):
                trp = tps.tile([128, 128], F16, name=f"trp{t}_{i}", tag="trp")
                nc.tensor.transpose(trp[:, :], h_new[:, 128 * i:128 * (i + 1)],
                                    idt[:, :])
                dst = hT_new[:, 128 * i:128 * (i + 1)]
                if eng == "act":
                    nc.scalar.activation(dst, trp[:, :], Copy)
                else:
                    nc.vector.tensor_copy(dst, trp[:, :])

            def emit_fc(t, hT, ms):
                for m in ms:
                    lhsT = hT[:, m * 64:(m + 1) * 64]
                    for g in range(2):
                        nc.tensor.matmul(
                            fcs[t][64 * g:64 * (g + 1), :], lhsT,
                            f1[:, m * 768 + g * 384: m * 768 + g * 384 + 384],
                            start=(m == 0), stop=(m == 7))

            def emit_fc_out(t):
                st = tmp.tile([128, 384], F32, name=f"st{t}", tag="st")
                nc.vector.tensor_add(st[:], fcs[t][:], cf[:])
                nc.sync.dma_start(out_d[t * 128:(t + 1) * 128, :], st[:])

            fcs = {}

            h_prev, hT_prev = h0, hT0
            for t in range(T):
                if t > 0:
                    # G matmuls: pre_r, h_n full-N; i_n, pre_z in N=256
                    # halves interleaved so each half's gate chain starts
                    # as early as possible
                    skip_gc = t > 1
                    emit_G_bank(hT_prev, 0, 0, 512, skip_gc)
                    emit_G_bank(hT_prev, 3, 0, 512, skip_gc)
                    emit_G_bank(hT_prev, 2, 0, 256, skip_gc)
                    emit_G_bank(hT_prev, 1, 0, 256, skip_gc)
                    emit_G_bank(hT_prev, 2, 256, 512, skip_gc)
                    emit_G_bank(hT_prev, 1, 256, 512, skip_gc)
                    h_new = emit_gates(
                        t, h_prev,
                        gb[0][:, :],
                        lambda hf: gb[1][:, 256 * hf:256 * (hf + 1)],
                        lambda hf: gb[2][:, 256 * hf:256 * (hf + 1)],
                        gb[3][:, :],
                        init_after_r=(0 if t + 1 < T else None))
                else:
                    h_new = emit_gates(
                        t, h_prev,
                        g0[:, 0:512],
                        lambda hf: g0[:, 512 + 256 * hf:512 + 256 * (hf + 1)],
                        lambda hf: g0[:, 1024 + 256 * hf:1024 + 256 * (hf + 1)],
                        g0[:, 1536:2048])

                hT_new = state.tile([128, 8 * 64], F16, name=f"hT{t}", tag="hT")

                if t == 0:
                    if T > 1:
                        emit_init_mm([0, 3, 2, 1])

                if t > 0:
                    fcs[t - 1] = fps.tile([128, 384], F32, name=f"fcp{t-1}",
                                          tag="fcp")
                    emit_fc(t - 1, hT_prev, [0, 1, 2, 3])
                emit_transpose(t, h_new, hT_new, 0, eng="act")
                emit_transpose(t, h_new, hT_new, 1, eng="act")
                if t > 0:
                    emit_fc(t - 1, hT_prev, [4, 5, 6, 7])
                emit_transpose(t, h_new, hT_new, 2, eng="dve")
                emit_transpose(t, h_new, hT_new, 3, eng="dve")
                # remaining next-step constants into PSUM (ACT queue tail)
                if t > 0 and t + 1 < T:
                    emit_init_eng(3)
                    emit_init_eng(1)
                    emit_init_eng(2)
                if t > 0:
                    emit_fc_out(t - 1)

                h_prev, hT_prev = h_new, hT_new

            fcs[T - 1] = fps.tile([128, 384], F32, name=f"fcp{T-1}", tag="fcp")
            emit_fc(T - 1, hT_prev, list(range(8)))
            emit_fc_out(T - 1)

    nc.compile()
    return nc


def _hi_lo(x):
    hi = x.astype(np.float16)
    lo = (x - hi.astype(np.float32)).astype(np.float16)
    return hi, lo


def kernel(src, hidden, W_ih, W_hh, b_ih, b_hh, fc_W, fc_b, output_len):
    from concourse import bass_utils

    T = int(output_len)
    src = np.asarray(src, np.float32)
    hidden = np.asarray(hidden, np.float32)
    W_ih = np.asarray(W_ih, np.float32)
    W_hh = np.asarray(W_hh, np.float32)
    b_ih = np.asarray(b_ih, np.float32)
    b_hh = np.asarray(b_hh, np.float32)
    fc_W = np.asarray(fc_W, np.float32)
    fc_b = np.asarray(fc_b, np.float32)

    ctx = hidden[0]          # [B, H]
    h0 = hidden[0]
    x0 = src[0]              # [B, OUT]

    # ---- host weight folding (fp32) ----
    M1 = W_ih @ fc_W[:, :H]          # [3H, H]
    M2 = W_ih @ fc_W[:, H:]          # [3H, H]
    P_r = M1[0:H] + W_hh[0:H]
    P_z = M1[H:2 * H] + W_hh[H:2 * H]
    Wbig4 = np.concatenate([P_r, P_z, M1[2 * H:], W_hh[2 * H:]], axis=0)  # [4096, H]
    F1 = fc_W[:, :H]                 # [OUT, H]

    CONST = ctx @ M2.T + (fc_b @ W_ih.T + b_ih)     # [B, 3H]
    c_r = CONST[:, 0:H] + b_hh[0:H]
    c_z = CONST[:, H:2 * H] + b_hh[H:2 * H]
    c_in = CONST[:, 2 * H:]
    c_hn = np.broadcast_to(b_hh[2 * H:], (BATCH, H)).astype(np.float32)
    CALL = np.stack([c_r, c_z, c_in, c_hn], axis=1)  # [B, 4, H]

    CF = ctx @ fc_W[:, H:].T + fc_b                  # [B, OUT]

    gi0 = x0 @ W_ih.T + b_ih
    gh0 = h0 @ W_hh.T + b_hh
    G0_parts = np.stack([gi0[:, :H] + gh0[:, :H],
                         gi0[:, H:2 * H] + gh0[:, H:2 * H],
                         gi0[:, 2 * H:],
                         gh0[:, 2 * H:]], axis=1)    # [B, 4, H]

    # ---- shared (replicated) tensors, K-chunk order m: dims offs(m)+p ----
    W4r = Wbig4.T.reshape(H, 4, 2, 512)              # [k, jj, g, c]
    W4s = np.empty((128, 8, 8, 512), np.float32)     # [p, m, cc, c]
    F1r = F1.T.reshape(H, 2, 384)                    # [k, g, c]
    F1s = np.empty((128, 8, 2, 384), np.float32)
    for m in range(8):
        o = _offs(m)
        W4s[:, m] = W4r[o:o + 128].reshape(128, 8, 512)
        F1s[:, m] = F1r[o:o + 128]
    W4s = W4s.reshape(128, 8 * 4096).astype(np.float16)
    F1s = F1s.reshape(128, 8 * 768).astype(np.float16)
    ID2 = np.concatenate([np.eye(64), np.eye(64)], axis=0).astype(np.float16)
    IDT = np.eye(128).astype(np.float16)

    key = T
    if key not in _BUILD_CACHE:
        _BUILD_CACHE[key] = _build(T)
    nc = _BUILD_CACHE[key]

    in_maps = []
    for c in range(NCORES):
        sl = slice(c * B, (c + 1) * B)
        # CINIT: [p, (jj*2+g)*512 + c]: p<64 hi, p>=64 lo of CALL[b, jj, 512g+c]
        call_c = CALL[sl].reshape(B, 4, 2, 512)      # [b, jj, g, c]
        hi, lo = _hi_lo(call_c)
        ci = np.concatenate([hi, lo], axis=0)        # [128, 4, 2, 512]
        ci = np.ascontiguousarray(ci).reshape(128, 8 * 512)

        # CST: [64g+b, jj*512 + c] = CALL[b, jj, 512g+c]  (fp32)
        cst = np.ascontiguousarray(
            call_c.transpose(2, 0, 1, 3)).reshape(128, 4 * 512)

        h0_c = h0[sl]
        H0S = np.concatenate([h0_c[:, :512], h0_c[:, 512:]], axis=0)
        # H0T[p, 128i + 64g + b] = h0[b, 512g + 128i + p]
        H0T = np.ascontiguousarray(
            h0_c.T.reshape(2, 4, 128, B).transpose(2, 1, 0, 3)).reshape(128, 8 * B)

        # G0: [64g+b, jj*512 + c] = G0_parts[b, jj, 512g+c]
        g0_c = G0_parts[sl].reshape(B, 4, 2, 512)    # [b, jj, g, c]
        G0s = np.ascontiguousarray(
            g0_c.transpose(2, 0, 1, 3)).reshape(128, 4 * 512)

        cf_c = CF[sl].reshape(B, 2, 384)             # [b, g, c]
        CFs = np.ascontiguousarray(cf_c.transpose(1, 0, 2)).reshape(128, 384)

        in_maps.append({
            "W4": W4s, "F1": F1s,
            "CINIT": np.ascontiguousarray(ci).astype(np.float16),
            "CST": cst.astype(np.float16),
            "IDENT2": ID2, "IDENTT": IDT,
            "H0S": np.ascontiguousarray(H0S).astype(np.float16),
            "H0T": H0T.astype(np.float16),
            "G0": G0s.astype(np.float32),
            "CF": CFs.astype(np.float32),
        })

    trace = bool(os.environ.get("GRU_TRACE"))
    res = bass_utils.run_bass_kernel_spmd(
        nc, in_maps, core_ids=list(range(NCORES)), trace=trace)
    if trace:
        kernel.last_exec_time_ns = res.exec_time_ns
        kernel.last_results = res

    outs = []
    for c in range(NCORES):
        o = res.results[c]["OUT"]                    # [T*128, 384]
        o = o.reshape(T, 2, B, 384).transpose(0, 2, 1, 3).reshape(T, B, OUT)
        outs.append(o)
    return np.concatenate(outs, axis=1)              # [T, BATCH, OUT]


# revision 18
# speedup vs baseline: 1.1836x; 1.1836x over previous
"""GRU decoder Bass kernel for Trainium2, data-parallel over batch on 8 cores.

Math refactoring (exactly equivalent to the reference up to fp assoc.):
  context = hidden[0] is constant across steps, and x_{t} = fc_out_{t-1} is
  linear in [h_t, context].  Folding fc into the input projection:
    gi_t = h_t @ M1.T + CONST          (M1 = W_ih @ fc_W[:, :H], t >= 1)
    gh_t = h_t @ W_hh.T + b_hh
  r/z gates add gi+gh, so P_r = M1_r + W_hh_r, P_z = M1_z + W_hh_z fuse into
  one [4096, 1024] weight:  G_t = h_t @ [P_r | P_z | M1_n | W_hh_n].T + C
  fc_out_t = h_{t+1} @ F1.T + CF with F1 = fc_W[:, :H].
  GRU update in "w-form": w = sigmoid(-pre_z) = 1-z, h' = h + w*(n - h).

Performance structure (v7):
  - per core B=64 rows, "split layout": [128 parts = 2 hidden-halves x 64
    batch, 512 free].  M=64 matmul pairs are col-tiled (auto tile_position
    (0,0)/(0,64)) and stream concurrently -> G = 16384 effective columns =
    6.9us/step, the fp16 1 col/cycle PE floor.
  - PSUM deps are tile-granular, so pre_z is accumulated into TWO separate
    half-banks (z_a cols 0-255, z_b cols 256-511); w = sigmoid(-pre_z)
    halves unblock right after each half-bank completes.
  - bank stream order pre_r, h_n, i_n, z_a, z_b; the loop-carried chain is
    [z_a done] -> w_a -> v_a -> h'_a -> transpose -> copy -> next G.
  - gate-bank constants are engine-written into PSUM in idle windows
    (has_written bits stay set from the previous step, so start=False MMs
    accumulate on top); no PE init matmuls per step.
  - fp16 state + intermediates; fc deferred one step fills the PE pipe
    during the gate tail; 4x 128x128 fp16 PE transposes; hT copies split
    ACT/DVE.  PE never idles long enough to re-throttle (HAM stays 2.4GHz).
"""
import os
import numpy as np

H = 1024
OUT = 768
BATCH = 512
NCORES = 8
B = BATCH // NCORES  # 64

_BUILD_CACHE = {}

# K-chunk m covers contraction dims offs(m) .. offs(m)+127
def _offs(m):
    return 128 * (m // 2) + 512 * (m % 2)


def _build(T: int):
    from contextlib import ExitStack
    from concourse import tile, mybir, bacc

    F16 = mybir.dt.float16
    F32 = mybir.dt.float32
    Sig = mybir.ActivationFunctionType.Sigmoid
    Tanh = mybir.ActivationFunctionType.Tanh
    Copy = mybir.ActivationFunctionType.Copy

    nc = bacc.Bacc("TRN2", target_bir_lowering=False, debug=False,
                   num_devices=NCORES)

    dram = {}
    def din(name, shape, dt):
        dram[name] = nc.dram_tensor(name, list(shape), dt, kind="ExternalInput").ap()
        return dram[name]

    w4_d = din("W4", [128, 8 * 4096], F16)
    f1_d = din("F1", [128, 8 * 768], F16)
    ci_d = din("CINIT", [128, 8 * 512], F16)
    cst_d = din("CST", [128, 4 * 512], F16)
    id2_d = din("IDENT2", [128, 64], F16)
    idt_d = din("IDENTT", [128, 128], F16)
    h0s_d = din("H0S", [128, 512], F16)
    h0t_d = din("H0T", [128, 8 * 64], F16)
    g0_d = din("G0", [128, 4 * 512], F32)
    cf_d = din("CF", [128, 384], F32)
    out_d = nc.dram_tensor("OUT", [T * 128, 384], F32, kind="ExternalOutput").ap()

    with tile.TileContext(nc) as tc:
        with ExitStack() as ctx:
            wpool = ctx.enter_context(tc.tile_pool(name="weights", bufs=1))
            state = ctx.enter_context(tc.tile_pool(name="state", bufs=2))
            tmp = ctx.enter_context(tc.tile_pool(name="tmp", bufs=2))
            gps = ctx.enter_context(tc.tile_pool(name="gpsum", bufs=1, space="PSUM"))
            fps = ctx.enter_context(tc.tile_pool(name="fpsum", bufs=1, space="PSUM"))
            tps = ctx.enter_context(tc.tile_pool(name="tpsum", bufs=2, space="PSUM"))

            w4 = wpool.tile([128, 8 * 4096], F16, name="w4")
            f1 = wpool.tile([128, 8 * 768], F16, name="f1")
            ci = wpool.tile([128, 8 * 512], F16, name="ci")
            cst = wpool.tile([128, 4 * 512], F16, name="cst")
            id2 = wpool.tile([128, 64], F16, name="id2")
            idt = wpool.tile([128, 128], F16, name="idt")
            g0 = wpool.tile([128, 4 * 512], F32, name="g0")
            cf = wpool.tile([128, 384], F32, name="cf")

            h0 = state.tile([128, 512], F16, name="h0", tag="h")
            hT0 = state.tile([128, 8 * 64], F16, name="hT0", tag="hT")

            for t_sb, t_d in ((w4, w4_d), (f1, f1_d), (ci, ci_d), (cst, cst_d),
                              (id2, id2_d), (idt, idt_d), (h0, h0s_d),
                              (hT0, h0t_d), (g0, g0_d), (cf, cf_d)):
                nc.sync.dma_start(t_sb[:], t_d[:])

            # gate PSUM banks: pre_r, h_n, i_n full; pre_z as two half-bank
            # tiles so each half's consumers unblock independently
            gb0 = gps.tile([128, 512], F32, name="gb0", tag="gb0")
            gb3 = gps.tile([128, 512], F32, name="gb3", tag="gb3")
            gb2 = gps.tile([128, 512], F32, name="gb2", tag="gb2")
            za = gps.tile([128, 256], F32, name="za", tag="za")
            zb = gps.tile([128, 256], F32, name="zb", tag="zb")

            def g_dest(jj, hf=0):
                # returns (tile, col offset within the logical 512-wide bank)
                if jj == 0: return gb0, 0
                if jj == 3: return gb3, 0
                if jj == 2: return gb2, 0
                return (za, 0) if hf == 0 else (zb, 256)

            def emit_init_mm(jjs):
                # PE-matmul constant init (prologue only): hi+lo fp16 rows
                for jj in jjs:
                    for g in range(2):
                        cc = jj * 2 + g
                        if jj == 1:
                            for hf in range(2):
                                dst, off = g_dest(1, hf)
                                nc.tensor.matmul(
                                    dst[64 * g:64 * (g + 1), :], id2[:, :],
                                    ci[:, cc * 512 + off: cc * 512 + off + 256],
                                    start=True, stop=False)
                        else:
                            dst, _ = g_dest(jj)
                            nc.tensor.matmul(
                                dst[64 * g:64 * (g + 1), :], id2[:, :],
                                ci[:, cc * 512:(cc + 1) * 512],
                                start=True, stop=False)

            def emit_init_eng(jj, eng, hf=None):
                # engine-written constants: has_written bits stay set from
                # the previous step's matmuls, so start=False MMs accumulate
                if jj == 1:
                    dst, off = g_dest(1, hf)
                    s = cst[:, 512 + off: 512 + off + 256]
                    ap = dst[:, :]
                else:
                    dst, _ = g_dest(jj)
                    s = cst[:, jj * 512:(jj + 1) * 512]
                    ap = dst[:, :]
                if eng == "act":
                    nc.scalar.activation(ap, s, Copy)
                else:
                    nc.vector.tensor_copy(ap, s)

            def emit_G_bank(hT, jj, hf, skip_gc):
                dst, off = g_dest(jj, hf)
                nn = 256 if jj == 1 else 512
                for m in range(8):
                    lhsT = hT[:, m * 64:(m + 1) * 64]
                    for g in range(2):
                        cc = jj * 2 + g
                        nc.tensor.matmul(
                            dst[64 * g:64 * (g + 1), :], lhsT,
                            w4[:, m * 4096 + cc * 512 + off:
                               m * 4096 + cc * 512 + off + nn],
                            start=False, stop=(m == 7),
                            skip_group_check=skip_gc)

            def emit_gates(t, h_prev, pr, pz_h, pin, phn, eng_init):
                r = tmp.tile([128, 512], F16, name=f"r{t}", tag="r")
                t1 = tmp.tile([128, 512], F16, name=f"t1{t}", tag="t1")
                t2 = tmp.tile([128, 512], F16, name=f"t2{t}", tag="t2")
                n = tmp.tile([128, 512], F16, name=f"n{t}", tag="n")
                dd = tmp.tile([128, 512], F16, name=f"d{t}", tag="d")
                w = tmp.tile([128, 512], F16, name=f"w{t}", tag="w")
                v = tmp.tile([128, 512], F16, name=f"v{t}", tag="v")
                h_new = state.tile([128, 512], F16, name=f"h{t}", tag="h")

                nc.scalar.activation(r[:], pr, Sig)
                if eng_init:
                    emit_init_eng(0, "dve")           # c_r: DVE idle window
                nc.vector.tensor_mul(t1[:], r[:], phn)
                if eng_init:
                    emit_init_eng(3, "dve")           # c_hn after t1
                for hf in range(2):
                    s = slice(256 * hf, 256 * (hf + 1))
                    nc.vector.tensor_add(t2[:, s], t1[:, s], pin(hf))
                    nc.scalar.activation(n[:, s], t2[:, s], Tanh)
                    nc.vector.tensor_sub(dd[:, s], n[:, s], h_prev[:, s])
                for hf in range(2):
                    s = slice(256 * hf, 256 * (hf + 1))
                    nc.scalar.activation(w[:, s], pz_h(hf), Sig, scale=-1.0)
                    nc.vector.tensor_mul(v[:, s], w[:, s], dd[:, s])
                    nc.vector.tensor_add(h_new[:, s], h_prev[:, s], v[:, s])
                return h_new

            def emit_transpose(t, h_new, hT_new, i, eng="dve"):
                trp = tps.tile([128, 128], F16, name=f"trp{t}_{i}", tag="trp")
                nc.tensor.transpose(trp[:, :], h_new[:, 128 * i:128 * (i + 1)],
                                    idt[:, :])
                dst = hT_new[:, 128 * i:128 * (i + 1)]
                if eng == "act":
                    nc.scalar.activation(dst, trp[:, :], Copy)
                else:
                    nc.vector.tensor_copy(dst, trp[:, :])

            def emit_fc(t, hT, ms):
                for m in ms:
                    lhsT = hT[:, m * 64:(m + 1) * 64]
                    for g in range(2):
                        nc.tensor.matmul(
                            fcs[t][64 * g:64 * (g + 1), :], lhsT,
                            f1[:, m * 768 + g * 384: m * 768 + g * 384 + 384],
                            start=(m == 0), stop=(m == 7))

            def emit_fc_out(t):
                st = tmp.tile([128, 384], F32, name=f"st{t}", tag="st")
                nc.vector.tensor_add(st[:], fcs[t][:], cf[:])
                nc.sync.dma_start(out_d[t * 128:(t + 1) * 128, :], st[:])

            fcs = {}

            h_prev, hT_prev = h0, hT0
            for t in range(T):
                eng_init = t > 0 and t + 1 < T
                if t > 0:
                    skip_gc = t > 1
                    emit_G_bank(hT_prev, 0, 0, skip_gc)
                    emit_G_bank(hT_prev, 3, 0, skip_gc)
                    emit_G_bank(hT_prev, 2, 0, skip_gc)
                    emit_G_bank(hT_prev, 1, 0, skip_gc)
                    emit_G_bank(hT_prev, 1, 1, skip_gc)
                    h_new = emit_gates(
                        t, h_prev,
                        gb0[:, :],
                        lambda hf: (za if hf == 0 else zb)[:, :],
                        lambda hf: gb2[:, 256 * hf:256 * (hf + 1)],
                        gb3[:, :], eng_init)
                else:
                    h_new = emit_gates(
                        t, h_prev,
                        g0[:, 0:512],
                        lambda hf: g0[:, 512 + 256 * hf:512 + 256 * (hf + 1)],
                        lambda hf: g0[:, 1024 + 256 * hf:1024 + 256 * (hf + 1)],
                        g0[:, 1536:2048], False)

                hT_new = state.tile([128, 8 * 64], F16, name=f"hT{t}", tag="hT")

                if t == 0:
                    if T > 1:
                        emit_init_mm([0, 3, 2, 1])

                if t > 0:
                    fcs[t - 1] = fps.tile([128, 384], F32, name=f"fcp{t-1}",
                                          tag="fcp")
                    emit_fc(t - 1, hT_prev, [0, 1, 2, 3])
                emit_transpose(t, h_new, hT_new, 0, eng="act")
                emit_transpose(t, h_new, hT_new, 1, eng="act")
                if t > 0:
                    emit_fc(t - 1, hT_prev, [4, 5, 6, 7])
                emit_transpose(t, h_new, hT_new, 2, eng="dve")
                emit_transpose(t, h_new, hT_new, 3, eng="dve")
                if eng_init:
                    emit_init_eng(1, "act", hf=0)     # c_z halves on ACT tail
                    emit_init_eng(1, "act", hf=1)
                if t > 0:
                    emit_fc_out(t - 1)
                if eng_init:
                    emit_init_eng(2, "dve")           # c_in: slack until next
                                                      # step's i_n matmuls

                h_prev, hT_prev = h_new, hT_new

            fcs[T - 1] = fps.tile([128, 384], F32, name=f"fcp{T-1}", tag="fcp")
            emit_fc(T - 1, hT_prev, list(range(8)))
            emit_fc_out(T - 1)

    nc.compile()
    return nc


def _hi_lo(x):
    hi = x.astype(np.float16)
    lo = (x - hi.astype(np.float32)).astype(np.float16)
    return hi, lo


def kernel(src, hidden, W_ih, W_hh, b_ih, b_hh, fc_W, fc_b, output_len):
    from concourse import bass_utils

    T = int(output_len)
    src = np.asarray(src, np.float32)
    hidden = np.asarray(hidden, np.float32)
    W_ih = np.asarray(W_ih, np.float32)
    W_hh = np.asarray(W_hh, np.float32)
    b_ih = np.asarray(b_ih, np.float32)
    b_hh = np.asarray(b_hh, np.float32)
    fc_W = np.asarray(fc_W, np.float32)
    fc_b = np.asarray(fc_b, np.float32)

    ctx = hidden[0]          # [B, H]
    h0 = hidden[0]
    x0 = src[0]              # [B, OUT]

    # ---- host weight folding (fp32) ----
    M1 = W_ih @ fc_W[:, :H]          # [3H, H]
    M2 = W_ih @ fc_W[:, H:]          # [3H, H]
    P_r = M1[0:H] + W_hh[0:H]
    P_z = M1[H:2 * H] + W_hh[H:2 * H]
    Wbig4 = np.concatenate([P_r, P_z, M1[2 * H:], W_hh[2 * H:]], axis=0)  # [4096, H]
    F1 = fc_W[:, :H]                 # [OUT, H]

    CONST = ctx @ M2.T + (fc_b @ W_ih.T + b_ih)     # [B, 3H]
    c_r = CONST[:, 0:H] + b_hh[0:H]
    c_z = CONST[:, H:2 * H] + b_hh[H:2 * H]
    c_in = CONST[:, 2 * H:]
    c_hn = np.broadcast_to(b_hh[2 * H:], (BATCH, H)).astype(np.float32)
    CALL = np.stack([c_r, c_z, c_in, c_hn], axis=1)  # [B, 4, H]

    CF = ctx @ fc_W[:, H:].T + fc_b                  # [B, OUT]

    gi0 = x0 @ W_ih.T + b_ih
    gh0 = h0 @ W_hh.T + b_hh
    G0_parts = np.stack([gi0[:, :H] + gh0[:, :H],
                         gi0[:, H:2 * H] + gh0[:, H:2 * H],
                         gi0[:, 2 * H:],
                         gh0[:, 2 * H:]], axis=1)    # [B, 4, H]

    # ---- shared (replicated) tensors, K-chunk order m: dims offs(m)+p ----
    W4r = Wbig4.T.reshape(H, 4, 2, 512)              # [k, jj, g, c]
    W4s = np.empty((128, 8, 8, 512), np.float32)     # [p, m, cc, c]
    F1r = F1.T.reshape(H, 2, 384)                    # [k, g, c]
    F1s = np.empty((128, 8, 2, 384), np.float32)
    for m in range(8):
        o = _offs(m)
        W4s[:, m] = W4r[o:o + 128].reshape(128, 8, 512)
        F1s[:, m] = F1r[o:o + 128]
    W4s = W4s.reshape(128, 8 * 4096).astype(np.float16)
    F1s = F1s.reshape(128, 8 * 768).astype(np.float16)
    ID2 = np.concatenate([np.eye(64), np.eye(64)], axis=0).astype(np.float16)
    IDT = np.eye(128).astype(np.float16)

    key = T
    if key not in _BUILD_CACHE:
        _BUILD_CACHE[key] = _build(T)
    nc = _BUILD_CACHE[key]

    in_maps = []
    for c in range(NCORES):
        sl = slice(c * B, (c + 1) * B)
        # CINIT: [p, (jj*2+g)*512 + c]: p<64 hi, p>=64 lo of CALL[b, jj, 512g+c]
        call_c = CALL[sl].reshape(B, 4, 2, 512)      # [b, jj, g, c]
        hi, lo = _hi_lo(call_c)
        ci = np.concatenate([hi, lo], axis=0)        # [128, 4, 2, 512]
        ci = np.ascontiguousarray(ci).reshape(128, 8 * 512)

        # CST: [64g+b, jj*512 + c] = CALL[b, jj, 512g+c]  (fp16)
        cst = np.ascontiguousarray(
            call_c.transpose(2, 0, 1, 3)).reshape(128, 4 * 512)

        h0_c = h0[sl]
        H0S = np.concatenate([h0_c[:, :512], h0_c[:, 512:]], axis=0)
        # H0T[p, 128i + 64g + b] = h0[b, 512g + 128i + p]
        H0T = np.ascontiguousarray(
            h0_c.T.reshape(2, 4, 128, B).transpose(2, 1, 0, 3)).reshape(128, 8 * B)

        # G0: [64g+b, jj*512 + c] = G0_parts[b, jj, 512g+c]
        g0_c = G0_parts[sl].reshape(B, 4, 2, 512)    # [b, jj, g, c]
        G0s = np.ascontiguousarray(
            g0_c.transpose(2, 0, 1, 3)).reshape(128, 4 * 512)

        cf_c = CF[sl].reshape(B, 2, 384)             # [b, g, c]
        CFs = np.ascontiguousarray(cf_c.transpose(1, 0, 2)).reshape(128, 384)

        in_maps.append({
            "W4": W4s, "F1": F1s,
            "CINIT": np.ascontiguousarray(ci).astype(np.float16),
            "CST": cst.astype(np.float16),
            "IDENT2": ID2, "IDENTT": IDT,
            "H0S": np.ascontiguousarray(H0S).astype(np.float16),
            "H0T": H0T.astype(np.float16),
            "G0": G0s.astype(np.float32),
            "CF": CFs.astype(np.float32),
        })

    trace = bool(os.environ.get("GRU_TRACE"))
    res = bass_utils.run_bass_kernel_spmd(
        nc, in_maps, core_ids=list(range(NCORES)), trace=trace)
    if trace:
        kernel.last_exec_time_ns = res.exec_time_ns
        kernel.last_results = res

    outs = []
    for c in range(NCORES):
        o = res.results[c]["OUT"]                    # [T*128, 384]
        o = o.reshape(T, 2, B, 384).transpose(0, 2, 1, 3).reshape(T, B, OUT)
        outs.append(o)
    return np.concatenate(outs, axis=1)              # [T, BATCH, OUT]
